# revision 1
# baseline (speedup 1.0000x reference)
"""Trainium2 Bass kernel for a single-token GQA decoder layer (B=64 batches),
tensor-parallel across 8 NeuronCores.

Contract: kernel(**inputs) takes the FULL fp32 inputs (as produced by the
reference setup_inputs) and returns the FULL [64, 1, 4096] fp32 output.

Sharding (TP-8): core c owns q heads [4c, 4c+4), kv head c, MLP rows
[1792c, 1792(c+1)); hidden dim replicated. One on-device AllReduce (bf16)
after the wo projection; the final down-proj partial sums are reduced on
host.

Perf design (DMA-roofline targeted):
- KV cache, q/k activations, softmax probs, attention weights (wqkv, wo)
  in fp8e4m3; MLP weights bf16 (output-critical).
- All weights host-packed into SBUF-image layouts and streamed with ~1-2MB
  DMAs (contiguous per-partition slabs).
- Attention per 4-batch group: QK with col-tiled PSUM bands, exp without
  max-subtraction (scores ~N(0,1); scale/bias folded into the activation),
  transpose+band-compaction+normalization fused into one matmul against a
  per-group scaled selection matrix, PV in the V-stationary orientation
  (produces oT directly, N=16 streams).
- Input DMAs ride nc.sync (SP HWDGE); output/collective-dependent DMAs ride
  nc.scalar (ACT HWDGE) so the AllReduce wait never stalls weight prefetch.
"""

import numpy as np

import concourse.bass as bass
import concourse.bacc as bacc
import concourse.mybir as mybir
import concourse.tile as tile
from concourse.bass_utils import run_bass_kernel_spmd

FP = mybir.dt.float32
BF = mybir.dt.bfloat16
F8 = mybir.dt.float8e4
AX = mybir.AxisListType
AF = mybir.ActivationFunctionType
ALU = mybir.AluOpType

NCORES = 8
B = 64                    # batch (= tokens, QLEN=1)
DIM = 4096
HD = 128
G = 4                     # local q heads per core
S = 2048                  # prefix length
IL = 14336 // NCORES      # local intermediate = 1792
QKV = (G + 2) * HD        # 768 local qkv rows
EPS = 1e-6
GRP = 4                   # batches per attention group
NGRP = B // GRP           # 16
CW = 448                  # MLP column chunk (IL = 4*448)
EXP_SCALE = 1.0 / float(np.sqrt(HD))
EXP_BIAS = -4.0
WQ_SCALE = 64.0           # host premultiplies wqkv by this (fp8 range)
WO_SCALE = 64.0           # host premultiplies wo by this
O_SCALE = 16.0            # device folds this into softmax normalization
OUT_UNSCALE = 1.0 / O_SCALE   # leaves x WO_SCALE for fp8 wire


def build_nc():
    nc = bacc.Bacc("TRN2", target_bir_lowering=False, debug=False,
                   num_devices=NCORES)

    # ---- DRAM I/O (per-core shards, host-prepped layouts) ----
    hs_d = nc.dram_tensor("hs", [B, DIM], FP, kind="ExternalInput")
    # per group t: [128, 4*2048 kT | 4*2048 v-seqmajor] fp8
    kv_d = nc.dram_tensor("kv", [NGRP, HD, 16384], F8, kind="ExternalInput")
    # 4 slabs of 8 j-blocks, each block [128, 768]
    wqkv_d = nc.dram_tensor("wqkvT", [4, HD, 8 * QKV], F8, kind="ExternalInput")
    biasc_d = nc.dram_tensor("biasc", [HD, 6], FP, kind="ExternalInput")
    qnw_d = nc.dram_tensor("qnw", [1, HD], FP, kind="ExternalInput")
    knw_d = nc.dram_tensor("knw", [1, HD], FP, kind="ExternalInput")
    ones_d = nc.dram_tensor("ones128", [HD, 1], FP, kind="ExternalInput")
    id64q_d = nc.dram_tensor("id64q", [64, 64], F8, kind="ExternalInput")
    id64b_d = nc.dram_tensor("id64b", [64, 64], BF, kind="ExternalInput")
    id128q_d = nc.dram_tensor("id128q", [128, 128], F8, kind="ExternalInput")
    sel_d = nc.dram_tensor("sel", [HD, 16], F8, kind="ExternalInput")
    mask4_d = nc.dram_tensor("mask4", [GRP, HD], FP, kind="ExternalInput")
    ones14_d = nc.dram_tensor("ones14", [1, GRP], F8, kind="ExternalInput")
    # [128, 4 kk-blocks * 4096]
    wo_d = nc.dram_tensor("woT", [HD, 4 * DIM], F8, kind="ExternalInput")
    # slab s = c*8+jj: 4 j-blocks x [up CW | gate CW]
    ug_d = nc.dram_tensor("ugT", [32, HD, 4 * 2 * CW], BF, kind="ExternalInput")
    # slab 2n+h: 7 c-blocks x [128, 512]
    dn_d = nc.dram_tensor("downT", [16, HD, 7 * 512], BF, kind="ExternalInput")

    partial_d = nc.dram_tensor("partial", [B, DIM], BF, kind="ExternalOutput")
    res2_d = nc.dram_tensor("res2", [B, DIM], FP, kind="ExternalOutput")

    with tile.TileContext(nc) as tc:
        with (
            tc.tile_pool(name="const", bufs=1) as constp,
            tc.tile_pool(name="sb", bufs=1) as sb,
            tc.tile_pool(name="kvs", bufs=2) as kvs,
            tc.tile_pool(name="wsl", bufs=9) as wsl,
            tc.tile_pool(name="att", bufs=2) as att,
            tc.tile_pool(name="small", bufs=4) as small,
            tc.tile_pool(name="ps_sc", bufs=2, space="PSUM") as ps_sc,
            tc.tile_pool(name="ps_stage", bufs=2, space="PSUM") as ps_stage,
            tc.tile_pool(name="ps_acc", bufs=2, space="PSUM") as ps_acc,
            tc.tile_pool(name="dram", bufs=1, space="DRAM") as dram,
        ):
            # ---- constants to SBUF (only id64q is needed early; the
            # rest are issued after the big input streams start) ----
            id64q = constp.tile([64, 64], F8, tag="id64q")
            nc.sync.dma_start(id64q[:], id64q_d[:])

            ebias = constp.tile([128, 1], FP, tag="ebias")
            nc.vector.memset(ebias[:], EXP_BIAS)

            hs = sb.tile([B, DIM], FP, tag="hs")
            nc.sync.dma_start(hs[:], hs_d[:])

            # wqkv slabs ride the shared weight-slab ring
            wq_t = []
            for s_ in range(4):
                wt = wsl.tile([HD, 8 * QKV], F8, tag="wsl")
                nc.sync.dma_start(wt[:], wqkv_d[s_])
                wq_t.append(wt)

            id64b = constp.tile([64, 64], BF, tag="id64b")
            nc.sync.dma_start(id64b[:], id64b_d[:])
            id128q = constp.tile([128, 128], F8, tag="id128q")
            nc.sync.dma_start(id128q[:], id128q_d[:])
            ones128 = constp.tile([HD, 1], FP, tag="ones")
            nc.sync.dma_start(ones128[:], ones_d[:])
            qnw = constp.tile([1, HD], FP, tag="qnw")
            nc.sync.dma_start(qnw[:], qnw_d[:])
            knw = constp.tile([1, HD], FP, tag="knw")
            nc.sync.dma_start(knw[:], knw_d[:])
            biasc = constp.tile([HD, 6], FP, tag="biasc")
            nc.sync.dma_start(biasc[:], biasc_d[:])
            sel = constp.tile([HD, 16], F8, tag="sel")
            nc.sync.dma_start(sel[:], sel_d[:])
            mask4 = constp.tile([GRP, HD], FP, tag="mask4")
            nc.sync.dma_start(mask4[:], mask4_d[:])
            ones14 = constp.tile([1, GRP], F8, tag="ones14")
            nc.sync.dma_start(ones14[:], ones14_d[:])

            # ================= helpers ==================================
            def rmsnorm_rstd(x_sb, tag):
                """rstd [64,1] fp32 for token-major x_sb [64, DIM]."""
                scr = sb.tile([B, DIM], F8, tag="x16")
                ssq = small.tile([B, 1], FP, tag=tag + "ssq")
                nc.scalar.activation(scr[:], x_sb[:], AF.Square,
                                     accum_out=ssq[:])
                t1 = small.tile([B, 1], FP, tag=tag + "t1")
                nc.vector.tensor_scalar(t1[:], ssq[:], 1.0 / DIM, EPS,
                                        op0=ALU.mult, op1=ALU.add)
                rcp = small.tile([B, 1], FP, tag=tag + "rcp")
                nc.vector.reciprocal(rcp[:], t1[:])
                rstd = small.tile([B, 1], FP, tag=tag + "rstd")
                nc.scalar.activation(rstd[:], rcp[:], AF.Sqrt)
                return rstd

            def transpose_rows(x_sb, ncols, dest, idm):
                """x_sb [64, ncols] -> dest [128, ncols//128*64]."""
                nch = ncols // 128
                for q in range(0, nch, 8):
                    hi = min(nch, q + 8)
                    stage = ps_stage.tile([128, 512], FP, tag="stage")
                    for j in range(q, hi):
                        nc.tensor.matmul(stage[:, (j - q) * 64:(j - q + 1) * 64],
                                         x_sb[:, j * 128:(j + 1) * 128],
                                         idm[:], start=True, stop=True)
                    nc.vector.tensor_copy(dest[:, q * 64:hi * 64],
                                          stage[:, 0:(hi - q) * 64])

            # ================= RMSNorm 1 + x^T (fp8) ====================
            rstd1 = rmsnorm_rstd(hs, "n1")
            x16 = sb.tile([B, DIM], F8, tag="x16")
            nc.vector.tensor_scalar_mul(x16[:], hs[:], rstd1[:])
            xT = sb.tile([128, B * DIM // 128], F8, tag="xT")   # [128, 2048]
            transpose_rows(x16, DIM, xT, id64q)

            # ================= QKV projection (fp8, x64 scaled) =========
            qkv_a = ps_acc.tile([B, 512], FP, tag="acc")
            qkv_b = ps_acc.tile([B, 256], FP, tag="acc")
            for j in range(32):
                wt = wq_t[j // 8]
                c0 = (j % 8) * QKV
                nc.tensor.matmul(qkv_a[:], xT[:, j * 64:(j + 1) * 64],
                                 wt[:, c0:c0 + 512], start=(j == 0),
                                 stop=(j == 31))
                nc.tensor.matmul(qkv_b[:], xT[:, j * 64:(j + 1) * 64],
                                 wt[:, c0 + 512:c0 + 768], start=(j == 0),
                                 stop=(j == 31))
            qkv_row = sb.tile([B, QKV], BF, tag="qkv_row")
            nc.vector.tensor_scalar_mul(qkv_row[:, 0:512], qkv_a[:],
                                        1.0 / WQ_SCALE)
            nc.vector.tensor_scalar_mul(qkv_row[:, 512:768], qkv_b[:],
                                        1.0 / WQ_SCALE)

            # transpose to [128 hd, 6*64] (fp32) and add bias
            qkvT = sb.tile([128, 6 * 64], FP, tag="qkvT")
            stage6 = ps_stage.tile([128, 512], FP, tag="stage")
            for c in range(6):
                nc.tensor.matmul(stage6[:, c * 64:(c + 1) * 64],
                                 qkv_row[:, c * 128:(c + 1) * 128],
                                 id64b[:], start=True, stop=True)
            for c in range(6):
                nc.vector.tensor_scalar_add(qkvT[:, c * 64:(c + 1) * 64],
                                            stage6[:, c * 64:(c + 1) * 64],
                                            biasc[:, c:c + 1])

            # ================= q/k rmsnorm (over partition dim HD) ======
            sq2 = sb.tile([128, 320], FP, tag="sq2")
            nc.scalar.activation(sq2[:], qkvT[:, 0:320], AF.Square)
            ss = ps_stage.tile([1, 320], FP, tag="stage")
            nc.tensor.matmul(ss[:], ones128[:], sq2[:], start=True, stop=True)
            t2 = small.tile([1, 320], FP, tag="t2", bufs=1)
            nc.vector.tensor_scalar(t2[:], ss[:], 1.0 / HD, EPS,
                                    op0=ALU.mult, op1=ALU.add)
            rcp2 = small.tile([1, 320], FP, tag="rcp2", bufs=1)
            nc.vector.reciprocal(rcp2[:], t2[:])
            rstd2 = small.tile([1, 320], FP, tag="rstd2", bufs=1)
            nc.scalar.activation(rstd2[:], rcp2[:], AF.Sqrt)

            bq = ps_stage.tile([128, 256], FP, tag="stage")
            nc.tensor.matmul(bq[:], qnw[:], rstd2[0:1, 0:256],
                             start=True, stop=True)
            qn = sb.tile([128, 256], F8, tag="qn")
            nc.vector.tensor_tensor(qn[:], qkvT[:, 0:256], bq[:], op=ALU.mult)
            bk = ps_stage.tile([128, 64], FP, tag="stage")
            nc.tensor.matmul(bk[:], knw[:], rstd2[0:1, 256:320],
                             start=True, stop=True)
            kn = sb.tile([128, 64], F8, tag="kn")
            nc.vector.tensor_tensor(kn[:], qkvT[:, 256:320], bk[:], op=ALU.mult)

            # v_new: v16 [128 hd, 64 tok] fp8, then per-group rows
            # vnewg [4, 16*128]: [b, t*128+d] = v_new[4t+b, d]
            v16 = sb.tile([128, 64], F8, tag="v16")
            nc.vector.tensor_copy(v16[:], qkvT[:, 320:384])
            vnewg = sb.tile([GRP, NGRP * HD], F8, tag="vnewg")
            for t in range(NGRP):
                vg_ps = ps_stage.tile([GRP, HD], FP, tag="stage")
                nc.tensor.matmul(vg_ps[:], v16[:, t * GRP:(t + 1) * GRP],
                                 id128q[:], start=True, stop=True)
                nc.vector.tensor_copy(vnewg[:, t * HD:(t + 1) * HD], vg_ps[:])

            # q slices ordered [128, tok, g] (col = g*64 + tok)
            qn_r = qn[:].rearrange("p (g t) -> p t g", g=G)

            # ================= attention ================================
            # group t = batches [4t, 4t+4); score rows (b,g) = 32b+g bands.
            # Two-stage software pipeline: while ACT computes exp(t), the
            # PE runs the transpose/PV of group t-1, so the PE never idles
            # long enough for HAM to re-throttle.
            oT = sb.tile([128, B * G], F8, tag="oT")   # col = 16t + 4b + g

            def qk_stage(t):
                kt = kvs.tile([HD, 8192], F8, tag="k", bufs=3)
                nc.sync.dma_start(kt[:], kv_d[t, :, 0:8192])
                vt = kvs.tile([HD, 8192], F8, tag="v", bufs=3)
                nc.sync.dma_start(vt[:], kv_d[t, :, 8192:16384])

                last = ps_acc.tile([128, 1], FP, tag="acc")
                nc.vector.memset(last[:], 0.0)
                sc_h = []
                for h in range(2):
                    sc = ps_sc.tile([128, 1024], FP, tag="sc")
                    if t == 0:
                        nc.vector.memset(sc[:], 0.0)
                    sc_h.append(sc)
                    for n in range(2):
                        for b in range(GRP):
                            bg = t * GRP + b
                            nc.tensor.matmul(
                                sc[32 * b:32 * b + 4, n * 512:(n + 1) * 512],
                                qn_r[:, bg],
                                kt[:, b * 2048 + (2 * h + n) * 512:
                                   b * 2048 + (2 * h + n + 1) * 512],
                                start=True, stop=True,
                                tile_position=(0, 32 * b))
                    if h == 0:
                        for b in range(GRP):
                            bg = t * GRP + b
                            nc.tensor.matmul(last[32 * b:32 * b + 4, 0:1],
                                             qn_r[:, bg], kn[:, bg:bg + 1],
                                             start=True, stop=True,
                                             tile_position=(0, 32 * b))

                # exp (no max-subtract: scores ~N(0,1); bias keeps fp8 range)
                p_sb = att.tile([128, S], F8, tag="p")
                s1a = small.tile([128, 1], FP, tag="s1a")
                s1b = small.tile([128, 1], FP, tag="s1b")
                nc.scalar.activation(p_sb[:, 0:1024], sc_h[0][:], AF.Exp,
                                     bias=ebias[:], scale=EXP_SCALE,
                                     accum_out=s1a[:])
                nc.scalar.activation(p_sb[:, 1024:2048], sc_h[1][:], AF.Exp,
                                     bias=ebias[:], scale=EXP_SCALE,
                                     accum_out=s1b[:])
                plf = small.tile([128, 1], F8, tag="plf")
                nc.scalar.activation(plf[:], last[:], AF.Exp,
                                     bias=ebias[:], scale=EXP_SCALE)
                return dict(t=t, vt=vt, p_sb=p_sb, s1a=s1a, s1b=s1b, plf=plf)

            def pv_stage(cx):
                t, vt, p_sb, plf = cx["t"], cx["vt"], cx["p_sb"], cx["plf"]
                # normalization scale (DVE; emitted here so the DVE queue
                # never stalls group t-1's work behind exp(t))
                stot = small.tile([128, 1], FP, tag="stot")
                nc.vector.tensor_tensor(stot[:], cx["s1a"][:], cx["s1b"][:],
                                        op=ALU.add)
                stot2 = small.tile([128, 1], FP, tag="stot2")
                nc.vector.tensor_tensor(stot2[:], stot[:], plf[:], op=ALU.add)
                t16 = small.tile([128, 1], FP, tag="t16")
                nc.vector.tensor_scalar_mul(t16[:], stot2[:], 1.0 / O_SCALE)
                rs = small.tile([128, 1], FP, tag="rs")
                nc.vector.reciprocal(rs[:], t16[:])   # = O_SCALE / sum

                # pT [128 seq, 16 (b,g)] per chunk via the selection
                # matrix (transpose + band-compaction in one matmul);
                # emitted first: lo chunks only need exp-lo, and the
                # pl/P4 chain below then hides under these on the PE.
                pT_ps = ps_stage.tile([128, 256], FP, tag="stage")
                for j in range(16):
                    nc.tensor.matmul(pT_ps[:, j * 16:(j + 1) * 16],
                                     p_sb[:, j * 128:(j + 1) * 128],
                                     sel[:], start=True, stop=True)
                pTa = att.tile([128, 256], F8, tag="pT")
                nc.vector.tensor_copy(pTa[:], pT_ps[:])

                # last-token band weights P4 [4, 128] (masked broadcast)
                pl_ps = ps_stage.tile([1, 128], FP, tag="stage")
                nc.tensor.matmul(pl_ps[:], plf[:], id128q[:],
                                 start=True, stop=True)
                plr = small.tile([1, 128], F8, tag="plr")
                nc.vector.tensor_copy(plr[:], pl_ps[:])
                bc4 = ps_stage.tile([GRP, HD], FP, tag="stage")
                nc.tensor.matmul(bc4[:], ones14[:], plr[:],
                                 start=True, stop=True)
                p4 = small.tile([GRP, HD], F8, tag="p4")
                nc.vector.tensor_tensor(p4[:], bc4[:], mask4[:], op=ALU.mult)

                # PV band-parallel: o_ps [128 bands, 128 hd]; consecutive
                # matmuls cycle the four PE column groups (concurrent).
                # P4 term first (start=True writes/clears every band row
                # once), then all PV matmuls accumulate.
                o_ps = ps_stage.tile([128, 128], FP, tag="stage")
                nc.tensor.matmul(o_ps[:], p4[:],
                                 vnewg[:, t * HD:(t + 1) * HD],
                                 start=True, stop=False,
                                 skip_group_check=True)
                for j in range(16):
                    for b in range(GRP):
                        nc.tensor.matmul(
                            o_ps[32 * b:32 * b + 4, :],
                            pTa[:, j * 16 + 4 * b:j * 16 + 4 * b + 4],
                            vt[:, b * 2048 + j * 128:
                               b * 2048 + (j + 1) * 128],
                            start=False, stop=(j == 15),
                            tile_position=(0, 32 * b),
                            skip_group_check=True)
                o_row = att.tile([128, 128], F8, tag="orow")
                nc.vector.tensor_scalar_mul(o_row[:], o_ps[:], rs[:])
                oT_ps = ps_stage.tile([128, 128], FP, tag="stage")
                nc.tensor.matmul(oT_ps[:], o_row[:], id128q[:],
                                 start=True, stop=True)
                oT_v = oT_ps[:].rearrange("p (b x) -> p b x", b=GRP)
                nc.vector.tensor_copy(
                    oT[:, t * 16:(t + 1) * 16].rearrange(
                        "p (b g) -> p b g", b=GRP),
                    oT_v[:, :, 0:G])

            cx_prev = None
            for t in range(NGRP):
                cx = qk_stage(t)
                if cx_prev is not None:
                    pv_stage(cx_prev)
                cx_prev = cx
            pv_stage(cx_prev)

            # ================= wo projection (fp8) ======================
            wo_sb = kvs.tile([HD, 4 * DIM], F8, tag="wo", bufs=1)
            nc.sync.dma_start(wo_sb[:], wo_d[:])
            oT_r = oT[:].rearrange("p (t b g) -> p g t b", t=NGRP, g=G)
            wo_out = sb.tile([B, DIM], F8, tag="wo_out")
            for n in range(8):
                wo_ps = ps_acc.tile([B, 512], FP, tag="acc")
                for kk in range(4):
                    nc.tensor.matmul(wo_ps[:], oT_r[:, kk],
                                     wo_sb[:, kk * DIM + n * 512:
                                           kk * DIM + (n + 1) * 512],
                                     start=(kk == 0), stop=(kk == 3))
                nc.vector.tensor_scalar_mul(wo_out[:, n * 512:(n + 1) * 512],
                                            wo_ps[:], OUT_UNSCALE)

            # ================= AllReduce (fp8, x64 scaled) ==============
            cc_in = dram.tile([B, DIM], F8)
            cc_out = dram.tile([B, DIM], F8)
            nc.scalar.dma_start(cc_in[:], wo_out[:])

            # prefetch the first MLP weight slabs on the scalar HWDGE
            # queue: it is idle while the collective runs, so these
            # stream during the AllReduce instead of after it
            ug_t = []
            for s_ in range(9):
                wt = wsl.tile([HD, 4 * 2 * CW], BF, tag="wsl")
                nc.scalar.dma_start(wt[:], ug_d[s_])
                ug_t.append(wt)

            nc.gpsimd.collective_compute(
                "AllReduce", ALU.add,
                replica_groups=[list(range(NCORES))],
                ins=[cc_in[:].opt()], outs=[cc_out[:].opt()],
            )

            ar = sb.tile([B, DIM], F8, tag="ar")
            nc.scalar.dma_start(ar[:], cc_out[:])
            hidden = hs
            nc.vector.scalar_tensor_tensor(hidden[:], ar[:], 1.0 / WO_SCALE,
                                           hs[:], op0=ALU.mult, op1=ALU.add)
            nc.scalar.dma_start(res2_d[:], hidden[:])

            # ================= RMSNorm 2 + MLP ==========================
            rstd2h = rmsnorm_rstd(hidden, "n2")
            h16 = sb.tile([B, DIM], BF, tag="h16")
            nc.vector.tensor_scalar_mul(h16[:], hidden[:], rstd2h[:])
            hT = sb.tile([128, B * DIM // 128], BF, tag="hT")
            transpose_rows(h16, DIM, hT, id64b)

            g_row = sb.tile([B, IL], BF, tag="g_row")
            gu_row = sb.tile([B, IL], BF, tag="gu_row")
            for c in range(4):
                up_ps = ps_acc.tile([B, CW], FP, tag="acc")
                gt_ps = ps_acc.tile([B, CW], FP, tag="acc")
                for jj in range(8):
                    s_ = c * 8 + jj
                    if s_ < 9:
                        wt = ug_t[s_]
                    else:
                        wt = wsl.tile([HD, 4 * 2 * CW], BF, tag="wsl")
                        nc.sync.dma_start(wt[:], ug_d[s_])
                    for jb in range(4):
                        j = jj * 4 + jb
                        nc.tensor.matmul(up_ps[:],
                                         hT[:, j * 64:(j + 1) * 64],
                                         wt[:, jb * 2 * CW:jb * 2 * CW + CW],
                                         start=(j == 0), stop=(j == 31))
                        nc.tensor.matmul(gt_ps[:],
                                         hT[:, j * 64:(j + 1) * 64],
                                         wt[:, jb * 2 * CW + CW:
                                               (jb + 1) * 2 * CW],
                                         start=(j == 0), stop=(j == 31))
                nc.scalar.activation(g_row[:, c * CW:(c + 1) * CW], gt_ps[:],
                                     AF.Silu)
                nc.vector.tensor_tensor(gu_row[:, c * CW:(c + 1) * CW],
                                        up_ps[:], g_row[:, c * CW:(c + 1) * CW],
                                        op=ALU.mult)

            guT = sb.tile([128, 14 * 64], BF, tag="guT")
            transpose_rows(gu_row, IL, guT, id64b)

            for n in range(8):
                wta = wsl.tile([HD, 7 * 512], BF, tag="wsl")
                nc.sync.dma_start(wta[:], dn_d[2 * n])
                wtb = wsl.tile([HD, 7 * 512], BF, tag="wsl")
                nc.sync.dma_start(wtb[:], dn_d[2 * n + 1])
                dn_ps = ps_acc.tile([B, 512], FP, tag="acc")
                for cc in range(14):
                    wt = wta if cc < 7 else wtb
                    nc.tensor.matmul(dn_ps[:], guT[:, cc * 64:(cc + 1) * 64],
                                     wt[:, (cc % 7) * 512:(cc % 7 + 1) * 512],
                                     start=(cc == 0), stop=(cc == 13))
                stg = small.tile([B, 512], BF, tag="ostg", bufs=2)
                nc.vector.tensor_copy(stg[:], dn_ps[:])
                nc.scalar.dma_start(partial_d[:, n * 512:(n + 1) * 512],
                                    stg[:])

    nc.compile()
    return nc


def shard_inputs(inputs):
    """Full fp32 inputs -> list of 8 per-core input maps (host prep)."""
    f32 = np.float32
    bf16 = mybir.dt.np(BF)
    f8 = mybir.dt.np(F8)
    hs = np.ascontiguousarray(inputs["hidden_states"].reshape(B, DIM), f32)
    wqkv = np.asarray(inputs["wqkv_w"], f32)
    wb = np.asarray(inputs["wqkv_b"], f32)
    wo = np.asarray(inputs["wo_w"], f32)
    up = np.asarray(inputs["up_w"], f32)
    gate = np.asarray(inputs["gate_w"], f32)
    down = np.asarray(inputs["down_w"], f32)
    qnorm = np.asarray(inputs["qnorm_w"], f32)
    knorm = np.asarray(inputs["knorm_w"], f32)
    iln = np.asarray(inputs["in_ln_w"], f32)
    pln = np.asarray(inputs["post_ln_w"], f32)
    kc = np.asarray(inputs["k_cache"], f32)   # [B, S, 8, HD]
    vc = np.asarray(inputs["v_cache"], f32)

    id64q = np.eye(64, dtype=f8)
    id64b = np.eye(64, dtype=bf16)
    id128q = np.eye(128, dtype=f8)
    ones128 = np.ones((HD, 1), f32)
    qnw = qnorm.reshape(1, HD).astype(f32)   # NOTE: 1/sqrt(HD) in EXP_SCALE
    knw = knorm.reshape(1, HD).astype(f32)
    # selection matrix [128 bands, 16]: SEL[32b+g, 4b+g] = 1
    sel = np.zeros((HD, 16), f8)
    for b in range(GRP):
        for g in range(G):
            sel[32 * b + g, 4 * b + g] = 1.0
    # last-token band mask [4, 128]: mask[b, 32b+g] = 1
    mask4 = np.zeros((GRP, HD), f32)
    for b in range(GRP):
        for g in range(G):
            mask4[b, 32 * b + g] = 1.0
    ones14 = np.ones((1, GRP), f8)

    H = 32
    maps = []
    for c in range(NCORES):
        wq = wqkv[c * G * HD:(c + 1) * G * HD]              # [512, DIM]
        wk = wqkv[H * HD + c * HD:H * HD + (c + 1) * HD]    # [128, DIM]
        wv = wqkv[(H + 8) * HD + c * HD:(H + 8) * HD + (c + 1) * HD]
        wloc = np.concatenate([wq, wk, wv], axis=0)         # [768, DIM]
        wqkvT = (wloc * iln[None, :] * WQ_SCALE).T.astype(f8)  # [DIM, 768]
        # slab images: [4, 128, 8*768]
        wq_img = np.ascontiguousarray(
            wqkvT.reshape(4, 8, HD, QKV).transpose(0, 2, 1, 3)
            .reshape(4, HD, 8 * QKV))
        bq = wb[c * G * HD:(c + 1) * G * HD]
        bk = wb[H * HD + c * HD:H * HD + (c + 1) * HD]
        bv = wb[(H + 8) * HD + c * HD:(H + 8) * HD + (c + 1) * HD]
        biasc = np.ascontiguousarray(
            np.concatenate([bq, bk, bv]).reshape(6, HD).T)  # [128, 6]

        # KV slabs: [16, 128, 16384] = [kT 4x2048 | v-seqmajor 4x2048]
        kT = kc[:, :, c, :].transpose(0, 2, 1).astype(f8)   # [B, HD, S]
        vsm = (vc[:, :, c, :].reshape(B, 16, 128, HD)
               .transpose(0, 2, 1, 3).reshape(B, HD, S).astype(f8))
        kv_img = np.empty((NGRP, HD, 16384), f8)
        for t in range(NGRP):
            for b in range(GRP):
                kv_img[t, :, b * S:(b + 1) * S] = kT[t * GRP + b]
                kv_img[t, :, 8192 + b * S:8192 + (b + 1) * S] = \
                    vsm[t * GRP + b]

        woT = (wo[:, c * G * HD:(c + 1) * G * HD].T * WO_SCALE).astype(f8)
        wo_img = np.ascontiguousarray(
            woT.reshape(4, HD, DIM).transpose(1, 0, 2).reshape(HD, 4 * DIM))

        upT = ((up[c * IL:(c + 1) * IL] * pln[None, :]).T).astype(bf16)
        gateT = ((gate[c * IL:(c + 1) * IL] * pln[None, :]).T).astype(bf16)
        # ug slabs [32, 128, 4*2*CW]: s = c*8+jj, blocks jb: [up CW|gate CW]
        ug_img = np.empty((32, HD, 4 * 2 * CW), bf16)
        for cch in range(4):
            for jj in range(8):
                for jb in range(4):
                    j = jj * 4 + jb
                    ug_img[cch * 8 + jj, :, jb * 2 * CW:jb * 2 * CW + CW] = \
                        upT[j * HD:(j + 1) * HD,
                            cch * CW:(cch + 1) * CW]
                    ug_img[cch * 8 + jj, :, jb * 2 * CW + CW:
                           (jb + 1) * 2 * CW] = \
                        gateT[j * HD:(j + 1) * HD, cch * CW:(cch + 1) * CW]

        downT = down[:, c * IL:(c + 1) * IL].T.astype(bf16)  # [IL, DIM]
        dn_img = np.empty((16, HD, 7 * 512), bf16)
        for n in range(8):
            for h2 in range(2):
                for cc2 in range(7):
                    dn_img[2 * n + h2, :, cc2 * 512:(cc2 + 1) * 512] = \
                        downT[(7 * h2 + cc2) * HD:(7 * h2 + cc2 + 1) * HD,
                              n * 512:(n + 1) * 512]

        maps.append({
            "hs": hs, "kv": kv_img, "wqkvT": wq_img, "biasc": biasc,
            "qnw": qnw, "knw": knw, "ones128": ones128,
            "id64q": id64q, "id64b": id64b, "id128q": id128q,
            "sel": sel, "mask4": mask4, "ones14": ones14,
            "woT": wo_img, "ugT": ug_img, "downT": dn_img,
        })
    return maps


_NC = None


def _get_nc():
    global _NC
    if _NC is None:
        _NC = build_nc()
    return _NC


def run(inputs, **kw):
    nc = _get_nc()
    in_maps = shard_inputs(inputs)
    res = run_bass_kernel_spmd(nc, in_maps, list(range(NCORES)), **kw)
    out = res.results[0]["res2"].astype(np.float64)
    for c in range(NCORES):
        out = out + res.results[c]["partial"].astype(np.float64)
    return out.astype(np.float32).reshape(B, 1, DIM), res


def kernel(**inputs):
    out, _ = run(inputs)
    return out



# revision 3
# speedup vs baseline: 1.0923x; 1.0923x over previous
"""Trainium2 Bass kernel for a single-token GQA decoder layer (B=64 batches),
tensor-parallel across 8 NeuronCores.

Contract: kernel(**inputs) takes the FULL fp32 inputs (as produced by the
reference setup_inputs) and returns the FULL [64, 1, 4096] fp32 output.

Sharding (TP-8): core c owns q heads [4c, 4c+4), kv head c, MLP rows
[1792c, 1792(c+1)); hidden dim replicated. One on-device AllReduce (fp8)
after the wo projection; the final down-proj partial sums are reduced on
host.

Perf design v2 (single sequential weight stream):
- ALL large inputs ride ONE HWDGE ring (nc.sync) as 81 uniform 1MB slabs
  [128, 8192] fp8: [wqkv x4][K/V x32][wo x2][up/gate x29 bf16-bitcast]
  [down x14 bf16-bitcast]. One ring = strict FIFO data movement, so KV
  streaming is never starved by weight traffic, and the MLP weights
  stream during the AllReduce latency window using the ring buffers the
  attention phase just freed (ring depth 17 = 136KB/partition).
- Attention path in fp8 (KV cache, probs, wqkv, wo); MLP weights and
  activations bf16 (fp8 there busts the 2e-2 budget: the MLP output rms
  is 3.5x the residual rms).
- rmsnorm-2 scale is DEFERRED past the up/gate matmuls (applied via the
  Silu activation scale and a fused scalar_tensor_tensor), so the
  post-collective serial chain is just ar-copy -> residual add (bf16)
  -> PE transpose -> matmuls; the sum-of-squares runs concurrently on ACT.
- Attention per 4-batch group: QK with col-tiled PSUM bands, exp without
  max-subtraction (scores ~N(0,1); scale/bias folded into the activation),
  transpose+band-compaction fused into one matmul against a selection
  matrix, PV in the V-stationary orientation.
- Small/ordered DMAs (consts, hs, collective in/out, outputs) ride the
  ACT HWDGE ring (nc.scalar) so they never perturb the weight stream.
"""

import numpy as np

import concourse.bass as bass
import concourse.bacc as bacc
import concourse.mybir as mybir
import concourse.tile as tile
from concourse.bass_utils import run_bass_kernel_spmd

FP = mybir.dt.float32
BF = mybir.dt.bfloat16
F8 = mybir.dt.float8e4
AX = mybir.AxisListType
AF = mybir.ActivationFunctionType
ALU = mybir.AluOpType

NCORES = 8
B = 64                    # batch (= tokens, QLEN=1)
DIM = 4096
HD = 128
G = 4                     # local q heads per core
S = 2048                  # prefix length
IL = 14336 // NCORES      # local intermediate = 1792
QKV = (G + 2) * HD        # 768 local qkv rows
EPS = 1e-6
GRP = 4                   # batches per attention group
NGRP = B // GRP           # 16
CW = 448                  # MLP column chunk (IL = 4*448)
EXP_SCALE = 1.0 / float(np.sqrt(HD))
EXP_BIAS = -4.0
WQ_SCALE = 64.0           # host premultiplies wqkv by this (fp8 range)
WO_SCALE = 64.0           # host premultiplies wo by this
O_SCALE = 16.0            # device folds this into softmax normalization
OUT_UNSCALE = 1.0 / O_SCALE   # leaves x WO_SCALE for fp8 wire

# ---- stream slab indices ----
SLAB_W = 8192             # fp8 bytes per partition per slab
NSLAB_QKV = 4             # slabs 0-3: wqkvT, 8 j-blocks of 768 cols each
SLAB_KV0 = 4              # slabs 4..35: per group t, k at 4+2t, v at 5+2t
SLAB_WO = 36              # slabs 36-37: woT (kk 0,1 | kk 2,3)
SLAB_UG = 38              # slabs 38..66: up/gate bf16, 9 blocks of 448/slab
NSLAB_UG = 29
SLAB_DN = SLAB_UG + NSLAB_UG   # slabs 67..80: down bf16, 8 blocks of 512/slab
NSLAB_DN = 14
NSLAB = SLAB_DN + NSLAB_DN     # 81
RING = 16                 # stream ring depth (SBUF: 16 x 8KB/partition)


def build_nc():
    nc = bacc.Bacc("TRN2", target_bir_lowering=False, debug=False,
                   num_devices=NCORES)

    # ---- DRAM I/O ----
    strm_d = nc.dram_tensor("strm", [NSLAB, HD, SLAB_W], F8,
                            kind="ExternalInput")
    hs_d = nc.dram_tensor("hs", [B, DIM], FP, kind="ExternalInput")
    biasc_d = nc.dram_tensor("biasc", [HD, 6], FP, kind="ExternalInput")
    qnw_d = nc.dram_tensor("qnw", [1, HD], FP, kind="ExternalInput")
    knw_d = nc.dram_tensor("knw", [1, HD], FP, kind="ExternalInput")
    ones_d = nc.dram_tensor("ones128", [HD, 1], FP, kind="ExternalInput")
    id64q_d = nc.dram_tensor("id64q", [64, 64], F8, kind="ExternalInput")
    id64b_d = nc.dram_tensor("id64b", [64, 64], BF, kind="ExternalInput")
    id128q_d = nc.dram_tensor("id128q", [128, 128], F8, kind="ExternalInput")
    sel_d = nc.dram_tensor("sel", [HD, 16], F8, kind="ExternalInput")
    mask4_d = nc.dram_tensor("mask4", [GRP, HD], FP, kind="ExternalInput")
    ones14_d = nc.dram_tensor("ones14", [1, GRP], F8, kind="ExternalInput")

    partial_d = nc.dram_tensor("partial", [B, DIM], BF, kind="ExternalOutput")
    res2_d = nc.dram_tensor("res2", [B, DIM], BF, kind="ExternalOutput")

    with tile.TileContext(nc) as tc:
        with (
            tc.tile_pool(name="const", bufs=1) as constp,
            tc.tile_pool(name="sb", bufs=1) as sb,
            tc.tile_pool(name="strm", bufs=RING) as strmp,
            tc.tile_pool(name="att", bufs=2) as att,
            tc.tile_pool(name="small", bufs=4) as small,
            tc.tile_pool(name="ps_sc", bufs=2, space="PSUM") as ps_sc,
            tc.tile_pool(name="ps_stage", bufs=2, space="PSUM") as ps_stage,
            tc.tile_pool(name="ps_acc", bufs=2, space="PSUM") as ps_acc,
            tc.tile_pool(name="dram", bufs=1, space="DRAM") as dram,
        ):
            # ---- the weight/KV stream: strict-order ring on nc.sync ----
            next_slab = [0]

            def slab(i):
                assert i == next_slab[0], (i, next_slab[0])
                t = strmp.tile([HD, SLAB_W], F8, tag="strm")
                nc.sync.dma_start(t[:], strm_d[i])
                next_slab[0] += 1
                return t

            # ---- consts + hs on the scalar (ACT) HWDGE ring ----
            id64q = constp.tile([64, 64], F8, tag="id64q")
            nc.scalar.dma_start(id64q[:], id64q_d[:])
            id64b = constp.tile([64, 64], BF, tag="id64b")
            nc.scalar.dma_start(id64b[:], id64b_d[:])
            id128q = constp.tile([128, 128], F8, tag="id128q")
            nc.scalar.dma_start(id128q[:], id128q_d[:])
            ones128 = constp.tile([HD, 1], FP, tag="ones")
            nc.scalar.dma_start(ones128[:], ones_d[:])
            qnw = constp.tile([1, HD], FP, tag="qnw")
            nc.scalar.dma_start(qnw[:], qnw_d[:])
            knw = constp.tile([1, HD], FP, tag="knw")
            nc.scalar.dma_start(knw[:], knw_d[:])
            biasc = constp.tile([HD, 6], FP, tag="biasc")
            nc.scalar.dma_start(biasc[:], biasc_d[:])
            sel = constp.tile([HD, 16], F8, tag="sel")
            nc.scalar.dma_start(sel[:], sel_d[:])
            mask4 = constp.tile([GRP, HD], FP, tag="mask4")
            nc.scalar.dma_start(mask4[:], mask4_d[:])
            ones14 = constp.tile([1, GRP], F8, tag="ones14")
            nc.scalar.dma_start(ones14[:], ones14_d[:])

            ebias = constp.tile([128, 1], FP, tag="ebias")
            nc.vector.memset(ebias[:], EXP_BIAS)

            hs = sb.tile([B, DIM], FP, tag="hs")
            nc.scalar.dma_start(hs[:], hs_d[:])

            # wqkv slabs lead the stream
            wq_t = [slab(i) for i in range(NSLAB_QKV)]

            # ================= helpers ==================================
            def rmsnorm_rstd(x_sb, tag):
                """rstd [64,1] fp32 for token-major x_sb [64, DIM]."""
                scr = sb.tile([B, DIM], F8, tag="x16")
                ssq = small.tile([B, 1], FP, tag=tag + "ssq")
                nc.scalar.activation(scr[:], x_sb[:], AF.Square,
                                     accum_out=ssq[:])
                t1 = small.tile([B, 1], FP, tag=tag + "t1")
                nc.vector.tensor_scalar(t1[:], ssq[:], 1.0 / DIM, EPS,
                                        op0=ALU.mult, op1=ALU.add)
                rcp = small.tile([B, 1], FP, tag=tag + "rcp")
                nc.vector.reciprocal(rcp[:], t1[:])
                rstd = small.tile([B, 1], FP, tag=tag + "rstd")
                nc.scalar.activation(rstd[:], rcp[:], AF.Sqrt)
                return rstd

            def transpose_rows(x_sb, ncols, dest, idm):
                """x_sb [64, ncols] -> dest [128, ncols//128*64]."""
                nch = ncols // 128
                for q in range(0, nch, 8):
                    hi = min(nch, q + 8)
                    stage = ps_stage.tile([128, 512], FP, tag="stage")
                    for j in range(q, hi):
                        nc.tensor.matmul(stage[:, (j - q) * 64:(j - q + 1) * 64],
                                         x_sb[:, j * 128:(j + 1) * 128],
                                         idm[:], start=True, stop=True)
                    nc.vector.tensor_copy(dest[:, q * 64:hi * 64],
                                          stage[:, 0:(hi - q) * 64])

            # ================= RMSNorm 1 + x^T (fp8) ====================
            rstd1 = rmsnorm_rstd(hs, "n1")
            x16 = sb.tile([B, DIM], F8, tag="x16")
            nc.vector.tensor_scalar_mul(x16[:], hs[:], rstd1[:])
            xT = sb.tile([128, B * DIM // 128], F8, tag="xT")   # [128, 2048]
            transpose_rows(x16, DIM, xT, id64q)

            # ================= QKV projection (fp8, x64 scaled) =========
            qkv_a = ps_acc.tile([B, 512], FP, tag="acc")
            qkv_b = ps_acc.tile([B, 256], FP, tag="acc")
            for j in range(32):
                wt = wq_t[j // 8]
                c0 = (j % 8) * QKV
                nc.tensor.matmul(qkv_a[:], xT[:, j * 64:(j + 1) * 64],
                                 wt[:, c0:c0 + 512], start=(j == 0),
                                 stop=(j == 31))
                nc.tensor.matmul(qkv_b[:], xT[:, j * 64:(j + 1) * 64],
                                 wt[:, c0 + 512:c0 + 768], start=(j == 0),
                                 stop=(j == 31))
            qkv_row = sb.tile([B, QKV], BF, tag="qkv_row")
            nc.vector.tensor_scalar_mul(qkv_row[:, 0:512], qkv_a[:],
                                        1.0 / WQ_SCALE)
            nc.vector.tensor_scalar_mul(qkv_row[:, 512:768], qkv_b[:],
                                        1.0 / WQ_SCALE)

            # transpose to [128 hd, 6*64] (fp32) and add bias
            qkvT = sb.tile([128, 6 * 64], FP, tag="qkvT")
            stage6 = ps_stage.tile([128, 512], FP, tag="stage")
            for c in range(6):
                nc.tensor.matmul(stage6[:, c * 64:(c + 1) * 64],
                                 qkv_row[:, c * 128:(c + 1) * 128],
                                 id64b[:], start=True, stop=True)
            for c in range(6):
                nc.vector.tensor_scalar_add(qkvT[:, c * 64:(c + 1) * 64],
                                            stage6[:, c * 64:(c + 1) * 64],
                                            biasc[:, c:c + 1])

            # ================= q/k rmsnorm (over partition dim HD) ======
            sq2 = sb.tile([128, 320], FP, tag="sq2")
            nc.scalar.activation(sq2[:], qkvT[:, 0:320], AF.Square)
            ss = ps_stage.tile([1, 320], FP, tag="stage")
            nc.tensor.matmul(ss[:], ones128[:], sq2[:], start=True, stop=True)
            t2 = small.tile([1, 320], FP, tag="t2", bufs=1)
            nc.vector.tensor_scalar(t2[:], ss[:], 1.0 / HD, EPS,
                                    op0=ALU.mult, op1=ALU.add)
            rcp2 = small.tile([1, 320], FP, tag="rcp2", bufs=1)
            nc.vector.reciprocal(rcp2[:], t2[:])
            rstd2 = small.tile([1, 320], FP, tag="rstd2", bufs=1)
            nc.scalar.activation(rstd2[:], rcp2[:], AF.Sqrt)

            bq = ps_stage.tile([128, 256], FP, tag="stage")
            nc.tensor.matmul(bq[:], qnw[:], rstd2[0:1, 0:256],
                             start=True, stop=True)
            qn = sb.tile([128, 256], F8, tag="qn")
            nc.vector.tensor_tensor(qn[:], qkvT[:, 0:256], bq[:], op=ALU.mult)
            bk = ps_stage.tile([128, 64], FP, tag="stage")
            nc.tensor.matmul(bk[:], knw[:], rstd2[0:1, 256:320],
                             start=True, stop=True)
            kn = sb.tile([128, 64], F8, tag="kn")
            nc.vector.tensor_tensor(kn[:], qkvT[:, 256:320], bk[:], op=ALU.mult)

            # v_new: v16 [128 hd, 64 tok] fp8, then per-group rows
            # vnewg [4, 16*128]: [b, t*128+d] = v_new[4t+b, d]
            v16 = sb.tile([128, 64], F8, tag="v16")
            nc.vector.tensor_copy(v16[:], qkvT[:, 320:384])
            vnewg = sb.tile([GRP, NGRP * HD], F8, tag="vnewg")
            for t in range(NGRP):
                vg_ps = ps_stage.tile([GRP, HD], FP, tag="stage")
                nc.tensor.matmul(vg_ps[:], v16[:, t * GRP:(t + 1) * GRP],
                                 id128q[:], start=True, stop=True)
                nc.vector.tensor_copy(vnewg[:, t * HD:(t + 1) * HD], vg_ps[:])

            # q slices ordered [128, tok, g] (col = g*64 + tok)
            qn_r = qn[:].rearrange("p (g t) -> p t g", g=G)

            # ================= attention ================================
            # group t = batches [4t, 4t+4); score rows (b,g) = 32b+g bands.
            # Two-stage software pipeline: while ACT computes exp(t), the
            # PE runs the transpose/PV of group t-1.
            oT = sb.tile([128, B * G], F8, tag="oT")   # col = 16t + 4b + g

            def qk_stage(t):
                kt = slab(SLAB_KV0 + 2 * t)
                vt = slab(SLAB_KV0 + 2 * t + 1)

                last = ps_acc.tile([128, 1], FP, tag="acc")
                nc.vector.memset(last[:], 0.0)
                sc_h = []
                for h in range(2):
                    sc = ps_sc.tile([128, 1024], FP, tag="sc")
                    if t == 0:
                        nc.vector.memset(sc[:], 0.0)
                    sc_h.append(sc)
                    for n in range(2):
                        for b in range(GRP):
                            bg = t * GRP + b
                            nc.tensor.matmul(
                                sc[32 * b:32 * b + 4, n * 512:(n + 1) * 512],
                                qn_r[:, bg],
                                kt[:, b * 2048 + (2 * h + n) * 512:
                                   b * 2048 + (2 * h + n + 1) * 512],
                                start=True, stop=True,
                                tile_position=(0, 32 * b))
                    if h == 0:
                        for b in range(GRP):
                            bg = t * GRP + b
                            nc.tensor.matmul(last[32 * b:32 * b + 4, 0:1],
                                             qn_r[:, bg], kn[:, bg:bg + 1],
                                             start=True, stop=True,
                                             tile_position=(0, 32 * b))

                # exp (no max-subtract: scores ~N(0,1); bias keeps fp8 range)
                p_sb = att.tile([128, S], F8, tag="p")
                s1a = small.tile([128, 1], FP, tag="s1a")
                s1b = small.tile([128, 1], FP, tag="s1b")
                nc.scalar.activation(p_sb[:, 0:1024], sc_h[0][:], AF.Exp,
                                     bias=ebias[:], scale=EXP_SCALE,
                                     accum_out=s1a[:])
                nc.scalar.activation(p_sb[:, 1024:2048], sc_h[1][:], AF.Exp,
                                     bias=ebias[:], scale=EXP_SCALE,
                                     accum_out=s1b[:])
                plf = small.tile([128, 1], F8, tag="plf")
                nc.scalar.activation(plf[:], last[:], AF.Exp,
                                     bias=ebias[:], scale=EXP_SCALE)
                return dict(t=t, vt=vt, p_sb=p_sb, s1a=s1a, s1b=s1b, plf=plf)

            def pv_stage(cx):
                t, vt, p_sb, plf = cx["t"], cx["vt"], cx["p_sb"], cx["plf"]
                stot = small.tile([128, 1], FP, tag="stot")
                nc.vector.tensor_tensor(stot[:], cx["s1a"][:], cx["s1b"][:],
                                        op=ALU.add)
                stot2 = small.tile([128, 1], FP, tag="stot2")
                nc.vector.tensor_tensor(stot2[:], stot[:], plf[:], op=ALU.add)
                t16 = small.tile([128, 1], FP, tag="t16")
                nc.vector.tensor_scalar_mul(t16[:], stot2[:], 1.0 / O_SCALE)
                rs = small.tile([128, 1], FP, tag="rs")
                nc.vector.reciprocal(rs[:], t16[:])   # = O_SCALE / sum

                # pT [128 seq, 16 (b,g)] per chunk via the selection matrix
                pT_ps = ps_stage.tile([128, 256], FP, tag="stage")
                for j in range(16):
                    nc.tensor.matmul(pT_ps[:, j * 16:(j + 1) * 16],
                                     p_sb[:, j * 128:(j + 1) * 128],
                                     sel[:], start=True, stop=True)
                pTa = att.tile([128, 256], F8, tag="pT")
                nc.vector.tensor_copy(pTa[:], pT_ps[:])

                # last-token band weights P4 [4, 128] (masked broadcast)
                pl_ps = ps_stage.tile([1, 128], FP, tag="stage")
                nc.tensor.matmul(pl_ps[:], plf[:], id128q[:],
                                 start=True, stop=True)
                plr = small.tile([1, 128], F8, tag="plr")
                nc.vector.tensor_copy(plr[:], pl_ps[:])
                bc4 = ps_stage.tile([GRP, HD], FP, tag="stage")
                nc.tensor.matmul(bc4[:], ones14[:], plr[:],
                                 start=True, stop=True)
                p4 = small.tile([GRP, HD], F8, tag="p4")
                nc.vector.tensor_tensor(p4[:], bc4[:], mask4[:], op=ALU.mult)

                # PV band-parallel: o_ps [128 bands, 128 hd]
                o_ps = ps_stage.tile([128, 128], FP, tag="stage")
                nc.tensor.matmul(o_ps[:], p4[:],
                                 vnewg[:, t * HD:(t + 1) * HD],
                                 start=True, stop=False,
                                 skip_group_check=True)
                for j in range(16):
                    for b in range(GRP):
                        nc.tensor.matmul(
                            o_ps[32 * b:32 * b + 4, :],
                            pTa[:, j * 16 + 4 * b:j * 16 + 4 * b + 4],
                            vt[:, b * 2048 + j * 128:
                               b * 2048 + (j + 1) * 128],
                            start=False, stop=(j == 15),
                            tile_position=(0, 32 * b),
                            skip_group_check=True)
                o_row = att.tile([128, 128], F8, tag="orow")
                nc.vector.tensor_scalar_mul(o_row[:], o_ps[:], rs[:])
                oT_ps = ps_stage.tile([128, 128], FP, tag="stage")
                nc.tensor.matmul(oT_ps[:], o_row[:], id128q[:],
                                 start=True, stop=True)
                oT_v = oT_ps[:].rearrange("p (b x) -> p b x", b=GRP)
                nc.vector.tensor_copy(
                    oT[:, t * 16:(t + 1) * 16].rearrange(
                        "p (b g) -> p b g", b=GRP),
                    oT_v[:, :, 0:G])

            cx_prev = None
            for t in range(NGRP):
                cx = qk_stage(t)
                if cx_prev is not None:
                    pv_stage(cx_prev)
                cx_prev = cx
            pv_stage(cx_prev)

            # ================= wo projection (fp8) ======================
            wo_t = [slab(SLAB_WO), slab(SLAB_WO + 1)]
            oT_r = oT[:].rearrange("p (t b g) -> p g t b", t=NGRP, g=G)
            wo_out = sb.tile([B, DIM], F8, tag="wo_out")
            for n in range(8):
                wo_ps = ps_acc.tile([B, 512], FP, tag="acc")
                for kk in range(4):
                    ws = wo_t[kk // 2]
                    c0 = (kk % 2) * DIM + n * 512
                    nc.tensor.matmul(wo_ps[:], oT_r[:, kk],
                                     ws[:, c0:c0 + 512],
                                     start=(kk == 0), stop=(kk == 3))
                nc.vector.tensor_scalar_mul(wo_out[:, n * 512:(n + 1) * 512],
                                            wo_ps[:], OUT_UNSCALE)

            # ================= AllReduce (fp8, x64 scaled) ==============
            cc_in = dram.tile([B, DIM], F8)
            cc_out = dram.tile([B, DIM], F8)
            nc.scalar.dma_start(cc_in[:], wo_out[:])

            nc.gpsimd.collective_compute(
                "AllReduce", ALU.add,
                replica_groups=[list(range(NCORES))],
                ins=[cc_in[:].opt()], outs=[cc_out[:].opt()],
            )

            ar = sb.tile([B, DIM], F8, tag="ar")
            nc.scalar.dma_start(ar[:], cc_out[:])
            # hidden (bf16): residual + allreduced attention output
            hidden = sb.tile([B, DIM], BF, tag="hid")
            nc.vector.scalar_tensor_tensor(hidden[:], ar[:], 1.0 / WO_SCALE,
                                           hs[:], op0=ALU.mult, op1=ALU.add)
            nc.scalar.dma_start(res2_d[:], hidden[:])

            # ================= RMSNorm 2 (deferred) + MLP ===============
            # rstd2h computed on ACT concurrently with the PE transpose;
            # its scale is applied after the up/gate matmuls.
            rstd2h = rmsnorm_rstd(hidden, "n2")
            hT = sb.tile([128, B * DIM // 128], BF, tag="hT")
            transpose_rows(hidden, DIM, hT, id64b)

            ug_slabs = {}

            def ug_block(idx):
                s = SLAB_UG + idx // 9
                if s not in ug_slabs:
                    ug_slabs[s] = slab(s)
                return ug_slabs[s], (idx % 9) * CW

            g_row = sb.tile([B, IL], BF, tag="g_row")
            gu_row = sb.tile([B, IL], BF, tag="gu_row")
            for c in range(4):
                up_ps = ps_acc.tile([B, CW], FP, tag="acc")
                gt_ps = ps_acc.tile([B, CW], FP, tag="acc")
                for j in range(32):
                    su, cu = ug_block(c * 64 + j * 2)
                    sg, cg = ug_block(c * 64 + j * 2 + 1)
                    nc.tensor.matmul(up_ps[:],
                                     hT[:, j * 64:(j + 1) * 64],
                                     su[:].bitcast(BF)[:, cu:cu + CW],
                                     start=(j == 0), stop=(j == 31))
                    nc.tensor.matmul(gt_ps[:],
                                     hT[:, j * 64:(j + 1) * 64],
                                     sg[:].bitcast(BF)[:, cg:cg + CW],
                                     start=(j == 0), stop=(j == 31))
                # g = silu(rstd * gate_raw); gu = (up_raw * rstd) * g
                nc.scalar.activation(g_row[:, c * CW:(c + 1) * CW], gt_ps[:],
                                     AF.Silu, scale=rstd2h[:])
                nc.vector.scalar_tensor_tensor(
                    gu_row[:, c * CW:(c + 1) * CW], up_ps[:], rstd2h[:],
                    g_row[:, c * CW:(c + 1) * CW],
                    op0=ALU.mult, op1=ALU.mult)

            guT = sb.tile([128, 14 * 64], BF, tag="guT")
            transpose_rows(gu_row, IL, guT, id64b)

            dn_slabs = {}

            def dn_block(idx):
                s = SLAB_DN + idx // 8
                if s not in dn_slabs:
                    dn_slabs[s] = slab(s)
                return dn_slabs[s], (idx % 8) * 512

            for n in range(8):
                dn_ps = ps_acc.tile([B, 512], FP, tag="acc")
                for cc in range(14):
                    sd, col = dn_block(n * 14 + cc)
                    nc.tensor.matmul(dn_ps[:], guT[:, cc * 64:(cc + 1) * 64],
                                     sd[:].bitcast(BF)[:, col:col + 512],
                                     start=(cc == 0), stop=(cc == 13))
                stg = small.tile([B, 512], BF, tag="ostg", bufs=2)
                nc.vector.tensor_copy(stg[:], dn_ps[:])
                nc.scalar.dma_start(partial_d[:, n * 512:(n + 1) * 512],
                                    stg[:])

            assert next_slab[0] == NSLAB, next_slab[0]

    nc.compile()
    return nc


def shard_inputs(inputs):
    """Full fp32 inputs -> list of 8 per-core input maps (host prep)."""
    f32 = np.float32
    bf16 = mybir.dt.np(BF)
    f8 = mybir.dt.np(F8)
    hs = np.ascontiguousarray(inputs["hidden_states"].reshape(B, DIM), f32)
    wqkv = np.asarray(inputs["wqkv_w"], f32)
    wb = np.asarray(inputs["wqkv_b"], f32)
    wo = np.asarray(inputs["wo_w"], f32)
    up = np.asarray(inputs["up_w"], f32)
    gate = np.asarray(inputs["gate_w"], f32)
    down = np.asarray(inputs["down_w"], f32)
    qnorm = np.asarray(inputs["qnorm_w"], f32)
    knorm = np.asarray(inputs["knorm_w"], f32)
    iln = np.asarray(inputs["in_ln_w"], f32)
    pln = np.asarray(inputs["post_ln_w"], f32)
    kc = np.asarray(inputs["k_cache"], f32)   # [B, S, 8, HD]
    vc = np.asarray(inputs["v_cache"], f32)

    id64q = np.eye(64, dtype=f8)
    id64b = np.eye(64, dtype=bf16)
    id128q = np.eye(128, dtype=f8)
    ones128 = np.ones((HD, 1), f32)
    qnw = qnorm.reshape(1, HD).astype(f32)   # NOTE: 1/sqrt(HD) in EXP_SCALE
    knw = knorm.reshape(1, HD).astype(f32)
    # selection matrix [128 bands, 16]: SEL[32b+g, 4b+g] = 1
    sel = np.zeros((HD, 16), f8)
    for b in range(GRP):
        for g in range(G):
            sel[32 * b + g, 4 * b + g] = 1.0
    # last-token band mask [4, 128]: mask[b, 32b+g] = 1
    mask4 = np.zeros((GRP, HD), f32)
    for b in range(GRP):
        for g in range(G):
            mask4[b, 32 * b + g] = 1.0
    ones14 = np.ones((1, GRP), f8)

    H = 32
    maps = []
    for c in range(NCORES):
        strm = np.zeros((NSLAB, HD, SLAB_W), f8)

        # --- slabs 0-3: wqkvT images (8 j-blocks of 768 cols each) ---
        wq = wqkv[c * G * HD:(c + 1) * G * HD]              # [512, DIM]
        wk = wqkv[H * HD + c * HD:H * HD + (c + 1) * HD]    # [128, DIM]
        wv = wqkv[(H + 8) * HD + c * HD:(H + 8) * HD + (c + 1) * HD]
        wloc = np.concatenate([wq, wk, wv], axis=0)         # [768, DIM]
        wqkvT = (wloc * iln[None, :] * WQ_SCALE).T.astype(f8)  # [DIM, 768]
        strm[0:4, :, 0:8 * QKV] = (
            wqkvT.reshape(4, 8, HD, QKV).transpose(0, 2, 1, 3)
            .reshape(4, HD, 8 * QKV))

        bq = wb[c * G * HD:(c + 1) * G * HD]
        bk = wb[H * HD + c * HD:H * HD + (c + 1) * HD]
        bv = wb[(H + 8) * HD + c * HD:(H + 8) * HD + (c + 1) * HD]
        biasc = np.ascontiguousarray(
            np.concatenate([bq, bk, bv]).reshape(6, HD).T)  # [128, 6]

        # --- slabs 4..35: KV (k seq-transposed, v seq-major) ---
        kT = kc[:, :, c, :].transpose(0, 2, 1).astype(f8)   # [B, HD, S]
        vsm = (vc[:, :, c, :].reshape(B, 16, 128, HD)
               .transpose(0, 2, 1, 3).reshape(B, HD, S).astype(f8))
        for t in range(NGRP):
            for b in range(GRP):
                strm[SLAB_KV0 + 2 * t, :, b * S:(b + 1) * S] = kT[t * GRP + b]
                strm[SLAB_KV0 + 2 * t + 1, :, b * S:(b + 1) * S] = \
                    vsm[t * GRP + b]

        # --- slabs 36-37: woT ---
        woT = (wo[:, c * G * HD:(c + 1) * G * HD].T * WO_SCALE).astype(f8)
        wo_img = (woT.reshape(4, HD, DIM).transpose(1, 0, 2)
                  .reshape(HD, 4 * DIM))
        strm[SLAB_WO] = wo_img[:, 0:SLAB_W]
        strm[SLAB_WO + 1] = wo_img[:, SLAB_W:2 * SLAB_W]

        # --- slabs 38..66: up/gate bf16, block id = c*64+j*2+{0=up,1=gate},
        #     9 blocks of 448 bf16 cols per slab ---
        upT = ((up[c * IL:(c + 1) * IL] * pln[None, :]).T).astype(bf16)
        gateT = ((gate[c * IL:(c + 1) * IL] * pln[None, :]).T).astype(bf16)
        ug_bf = np.zeros((NSLAB_UG, HD, SLAB_W // 2), bf16)
        for cch in range(4):
            for j in range(32):
                for g, wT in ((0, upT), (1, gateT)):
                    idx = cch * 64 + j * 2 + g
                    s_, b_ = idx // 9, (idx % 9) * CW
                    ug_bf[s_, :, b_:b_ + CW] = \
                        wT[j * HD:(j + 1) * HD, cch * CW:(cch + 1) * CW]
        strm[SLAB_UG:SLAB_UG + NSLAB_UG] = \
            ug_bf.view(np.uint8).reshape(NSLAB_UG, HD, SLAB_W).view(f8)

        # --- slabs 67..80: down bf16, block id = n*14+cc, 8 of 512/slab ---
        downT = down[:, c * IL:(c + 1) * IL].T.astype(bf16)  # [IL, DIM]
        dn_bf = np.zeros((NSLAB_DN, HD, SLAB_W // 2), bf16)
        for n in range(8):
            for cc in range(14):
                idx = n * 14 + cc
                s_, b_ = idx // 8, (idx % 8) * 512
                dn_bf[s_, :, b_:b_ + 512] = \
                    downT[cc * HD:(cc + 1) * HD, n * 512:(n + 1) * 512]
        strm[SLAB_DN:SLAB_DN + NSLAB_DN] = \
            dn_bf.view(np.uint8).reshape(NSLAB_DN, HD, SLAB_W).view(f8)

        maps.append({
            "strm": strm, "hs": hs, "biasc": biasc,
            "qnw": qnw, "knw": knw, "ones128": ones128,
            "id64q": id64q, "id64b": id64b, "id128q": id128q,
            "sel": sel, "mask4": mask4, "ones14": ones14,
        })
    return maps


_NC = None


def _get_nc():
    global _NC
    if _NC is None:
        _NC = build_nc()
    return _NC


def run(inputs, **kw):
    nc = _get_nc()
    in_maps = shard_inputs(inputs)
    res = run_bass_kernel_spmd(nc, in_maps, list(range(NCORES)), **kw)
    out = res.results[0]["res2"].astype(np.float64)
    for c in range(NCORES):
        out = out + res.results[c]["partial"].astype(np.float64)
    return out.astype(np.float32).reshape(B, 1, DIM), res


def kernel(**inputs):
    out, _ = run(inputs)
    return out
